# revision 22
# baseline (speedup 1.0000x reference)
"""Trainium2 Bass kernel: ContinuousNormalizingFlowODE.

Reference computes, per Euler step (T-1 = 7 steps):
    dx  = MLP(x, t)            (tanh, D=64 -> H=512 -> H=512 -> D=64)
    dz  = 0.5*||dx||^2
    dl  = tr(J)  where J = d(MLP)/dx   (reference: 64 forward-mode JVPs)

Key algebraic identity used here: with
    h1 = tanh(x@W1a + b1 + t*w1t),  s1 = 1 - h1^2
    h2 = tanh(h1@W2 + b2),          s2 = 1 - h2^2
    J  = W1a' D1 W2 D2 W3   (D=diag(s))
    tr(J) = s1^T (M .* W2) s2   with  M = W1a^T @ W3^T   (precomputable!)
So the whole Jacobian-trace costs ONE extra [B,H]@[H,H] matmul per step
instead of 64 JVPs.

Device mapping: pure data parallelism (64 samples/core on 8 cores).
Feature-major layout (features on SBUF partitions, samples on the free
axis) so that weights are always the stationary matmul operand and no
on-chip transposes are needed.  Biases and the time scalar are folded
into the contraction dimension (augmented rows of W1) or into rank-1
(K=1) matmuls.  z and l accumulate across all steps directly in PSUM
via dt-scaled ones-vector matmuls.
"""

import sys

for _p in ("/opt/trn_rl_repo", "/opt/trn_rl_repo/concourse"):
    if _p not in sys.path:
        sys.path.insert(0, _p)

import numpy as np

B, D, H, T = 512, 64, 512, 8
NCORES = 8
PC = B // NCORES          # samples per core (64)
NH = 1                    # batch groups per core (1: per-matmul cost is
                          # fixed-overhead dominated, so bigger N wins)
HB = PC // NH             # samples per group (64)
NSTEP = T - 1
KC = H // 128             # feature chunks of the hidden dim (4)

_CACHE: dict = {}


def _build(ts: np.ndarray, repeats: int = 1, use_f32r: bool = False):
    import concourse.bacc as bacc
    import concourse.mybir as mybir
    import concourse.tile as tile

    f32 = mybir.dt.float32
    # float32r: single-pass PE streaming (fp32 needs 2 half-speed passes);
    # keeps ~13 mantissa bits -> end-to-end ~3e-5 rel err (numpy-verified)
    md = mybir.dt.float32r if use_f32r else f32
    AF = mybir.ActivationFunctionType
    ALU = mybir.AluOpType

    ts_f = [float(v) for v in ts]

    nc = bacc.Bacc("TRN2", target_bir_lowering=False, debug=False)

    d_xt0 = nc.dram_tensor("xt0", [D + 1, PC], md, kind="ExternalInput")
    # per-step augmented W1: rows 0..D-1 = W1a, row D = b1 + ts[s]*w1t
    d_w1a = nc.dram_tensor("w1aug", [D + 1, NSTEP * H], md, kind="ExternalInput")
    d_w2r = nc.dram_tensor("w2r", [128, KC * H], md, kind="ExternalInput")
    d_ar = nc.dram_tensor("ar", [128, KC * H], md, kind="ExternalInput")
    d_w3r = nc.dram_tensor("w3r", [128, KC * D], md, kind="ExternalInput")
    d_b2 = nc.dram_tensor("b2row", [1, H], md, kind="ExternalInput")
    d_b3 = nc.dram_tensor("b3row", [1, D], md, kind="ExternalInput")
    d_dtz = nc.dram_tensor("dtz", [D, NSTEP], md, kind="ExternalInput")
    d_dtl = nc.dram_tensor("dtl", [128, NSTEP], md, kind="ExternalInput")
    d_xtout = nc.dram_tensor("xt_out", [D, PC], md, kind="ExternalOutput")
    d_zlout = nc.dram_tensor("zl_out", [1, 2 * PC], f32, kind="ExternalOutput")

    with tile.TileContext(nc) as tc:
        with (
            tc.tile_pool(name="consts", bufs=1) as consts,
            tc.tile_pool(name="acts", bufs=2) as acts,
            tc.tile_pool(name="pw", bufs=2, space="PSUM") as pw,
            tc.tile_pool(name="pdxp", bufs=1, space="PSUM") as pdxp,
            tc.tile_pool(name="pacc", bufs=1, space="PSUM") as pacc,
        ):
            # xtaug rows: 0..63 = x (feature-major), 64 = ones
            XTAUG = consts.tile([D + 1, PC], md, tag="xtaug")
            W1A = consts.tile([D + 1, NSTEP * H], md, tag="w1a")
            W2R = consts.tile([128, KC * H], md, tag="w2r")
            AR = consts.tile([128, KC * H], md, tag="ar")
            W3R = consts.tile([128, KC * D], md, tag="w3r")
            B2 = consts.tile([1, H], md, tag="b2")
            B3 = consts.tile([1, D], md, tag="b3")
            DTZ = consts.tile([D, NSTEP], md, tag="dtz")
            DTL = consts.tile([128, NSTEP], md, tag="dtl")
            ONES = consts.tile([1, 2 * PC], md, tag="ones")
            ZERO = consts.tile([1, 1], md, tag="zero")
            ZLS = consts.tile([1, 2 * PC], f32, tag="zls")

            nc.sync.dma_start(XTAUG[:], d_xt0.ap())
            nc.sync.dma_start(W1A[:], d_w1a.ap())
            nc.sync.dma_start(W3R[:], d_w3r.ap())
            nc.sync.dma_start(B2[:], d_b2.ap())
            nc.sync.dma_start(B3[:], d_b3.ap())
            nc.sync.dma_start(DTZ[:], d_dtz.ap())
            nc.sync.dma_start(DTL[:], d_dtl.ap())
            nc.sync.dma_start(W2R[:], d_w2r.ap())
            nc.sync.dma_start(AR[:], d_ar.ap())

            nc.gpsimd.memset(ONES[:], 1.0)
            nc.gpsimd.memset(ZERO[:], 0.0)

            ZL = pacc.tile([1, 2 * PC], f32, tag="zl")
            # Zero the whole z/l accumulator bank once (start=True also
            # clears the bank's has_written bits).  All later accumulating
            # matmuls use start=False: a start=True mid-stream would clear
            # the shared bank's has_written bits and drop other regions'
            # partial sums.
            nc.tensor.matmul(
                ZL[:], ZERO[:], ONES[:], start=True, stop=True,
                skip_group_check=True,
            )

            for s in range(NSTEP * repeats):
                s = s % NSTEP
                dt = ts_f[s + 1] - ts_f[s]
                for h in range(NH):
                    cs = slice(h * HB, (h + 1) * HB)

                    # ---- layer 1: p1 = W1a^T x + b1 + t*w1t (aug rows) ----
                    P1 = pw.tile([128, KC * HB], f32, tag="p1")
                    for c in range(KC):
                        nc.tensor.matmul(
                            P1[:, c * HB : (c + 1) * HB],
                            W1A[:, s * H + c * 128 : s * H + (c + 1) * 128],
                            XTAUG[:, cs],
                            start=True,
                            stop=True,
                        )
                    H1 = acts.tile([128, KC * HB], md, tag="h1")
                    nc.scalar.activation(H1[:], P1[:], AF.Tanh)
                    S1 = acts.tile([128, KC * HB], md, tag="s1")
                    nc.vector.tensor_tensor(S1[:], H1[:], H1[:], op=ALU.mult)
                    nc.vector.tensor_scalar(
                        S1[:], S1[:], -1.0, 1.0, ALU.mult, ALU.add
                    )

                    # ---- layer 2: p2 = W2^T h1 + b2 ----
                    P2 = pw.tile([128, KC * HB], f32, tag="p2")
                    for m in range(KC):
                        for k in range(KC):
                            nc.tensor.matmul(
                                P2[:, m * HB : (m + 1) * HB],
                                W2R[:, k * H + m * 128 : k * H + (m + 1) * 128],
                                H1[:, k * HB : (k + 1) * HB],
                                start=(k == 0),
                                stop=False,
                            )
                        nc.tensor.matmul(
                            P2[:, m * HB : (m + 1) * HB],
                            B2[:, m * 128 : (m + 1) * 128],
                            ONES[:, cs],
                            start=False,
                            stop=True,
                        )
                    H2 = acts.tile([128, KC * HB], md, tag="h2")
                    nc.scalar.activation(H2[:], P2[:], AF.Tanh)
                    S2 = acts.tile([128, KC * HB], md, tag="s2")
                    nc.vector.tensor_tensor(S2[:], H2[:], H2[:], op=ALU.mult)
                    nc.vector.tensor_scalar(
                        S2[:], S2[:], -1.0, 1.0, ALU.mult, ALU.add
                    )

                    # ---- trace bilinear form: u = A^T s1 ----
                    PU = pw.tile([128, KC * HB], f32, tag="pu")
                    for m in range(KC):
                        for k in range(KC):
                            nc.tensor.matmul(
                                PU[:, m * HB : (m + 1) * HB],
                                AR[:, k * H + m * 128 : k * H + (m + 1) * 128],
                                S1[:, k * HB : (k + 1) * HB],
                                start=(k == 0),
                                stop=(k == KC - 1),
                            )
                    WU = acts.tile([128, KC * HB], md, tag="wu")
                    nc.vector.tensor_tensor(WU[:], PU[:], S2[:], op=ALU.mult)

                    # ---- layer 3: dx = W3^T h2 + b3 ----
                    PDX = pdxp.tile([D, HB], f32, tag="pdx")
                    for k in range(KC):
                        nc.tensor.matmul(
                            PDX[:],
                            W3R[:, k * D : (k + 1) * D],
                            H2[:, k * HB : (k + 1) * HB],
                            start=(k == 0),
                            stop=False,
                        )
                    nc.tensor.matmul(
                        PDX[:], B3[:], ONES[:, cs], start=False, stop=True
                    )

                    # dx^2 on ScalarE: DVE can't read PSUM twice in one op,
                    # and Square shares the tanh table set (one table load).
                    SQ = acts.tile([D, HB], md, tag="sq")
                    nc.scalar.activation(SQ[:], PDX[:], AF.Square)

                    # ---- z += dt*0.5*sum(dx^2); l += dt*sum(u.*s2) ----
                    # (partition reduce via ones-matmul, accumulated in PSUM
                    # across all steps; dt folded into the lhsT columns)
                    nc.tensor.matmul(
                        ZL[:, h * HB : (h + 1) * HB],
                        DTZ[:, s : s + 1],
                        SQ[:],
                        start=False,
                        stop=(s == NSTEP - 1),
                        skip_group_check=True,
                    )
                    for k in range(KC):
                        nc.tensor.matmul(
                            ZL[:, PC + h * HB : PC + (h + 1) * HB],
                            DTL[:, s : s + 1],
                            WU[:, k * HB : (k + 1) * HB],
                            start=False,
                            stop=(s == NSTEP - 1 and k == KC - 1),
                            skip_group_check=True,
                        )

                    # ---- x += dt*dx (in place, feature-major) ----
                    nc.vector.scalar_tensor_tensor(
                        XTAUG[0:D, cs],
                        PDX[:],
                        dt,
                        XTAUG[0:D, cs],
                        op0=ALU.mult,
                        op1=ALU.add,
                    )

            nc.vector.tensor_copy(ZLS[:], ZL[:])
            nc.sync.dma_start(d_xtout.ap(), XTAUG[0:D, :])
            nc.sync.dma_start(d_zlout.ap(), ZLS[:])

    nc.compile()
    return nc


def _prepare_inputs(y0, ts, W1, b1, W2, b2, W3, b3):
    """Host-side packing: returns the per-core input maps."""
    f = np.float32
    y0 = np.ascontiguousarray(y0, f)
    ts = np.asarray(ts, f)
    W1 = np.asarray(W1, f)
    W2 = np.ascontiguousarray(W2, f)
    W3 = np.ascontiguousarray(W3, f)
    b1 = np.asarray(b1, f)
    b2 = np.asarray(b2, f)
    b3 = np.asarray(b3, f)

    W1a = W1[:D]                      # [64, 512]
    w1t = W1[D]                       # [512]
    A = (W1a.T @ W3.T) * W2           # [512, 512] trace bilinear matrix

    # per-step augmented W1: rows 0..D-1 = W1a, row D = b1 + ts[s]*w1t
    w1aug = np.empty((D + 1, NSTEP * H), f)
    for s in range(NSTEP):
        w1aug[:D, s * H : (s + 1) * H] = W1a
        w1aug[D, s * H : (s + 1) * H] = b1 + ts[s] * w1t
    w2r = np.ascontiguousarray(
        W2.reshape(KC, 128, H).transpose(1, 0, 2).reshape(128, KC * H)
    )
    ar = np.ascontiguousarray(
        A.reshape(KC, 128, H).transpose(1, 0, 2).reshape(128, KC * H)
    )
    w3r = np.ascontiguousarray(
        W3.reshape(KC, 128, D).transpose(1, 0, 2).reshape(128, KC * D)
    )
    dts = ts[1:] - ts[:-1]            # [7]
    dtz = np.ascontiguousarray(np.broadcast_to(0.5 * dts, (D, NSTEP)), f)
    dtl = np.ascontiguousarray(np.broadcast_to(dts, (128, NSTEP)), f)
    b2row = np.ascontiguousarray(b2[None, :])
    b3row = np.ascontiguousarray(b3[None, :])

    shared = {
        "w1aug": np.ascontiguousarray(w1aug),
        "w2r": w2r,
        "ar": ar,
        "w3r": w3r,
        "b2row": b2row,
        "b3row": b3row,
        "dtz": dtz,
        "dtl": dtl,
    }
    in_maps = []
    for c in range(NCORES):
        xt0 = np.empty((D + 1, PC), f)
        xt0[:D] = y0[c * PC : (c + 1) * PC].T
        xt0[D] = 1.0
        m = dict(shared)
        m["xt0"] = xt0
        in_maps.append(m)
    return in_maps


def _run(nc, in_maps, trace=False):
    from concourse import bass_utils

    res = bass_utils.run_bass_kernel_spmd(
        nc, in_maps, core_ids=list(range(NCORES)), trace=trace
    )
    return res


def _assemble(results):
    f = np.float32
    xf = np.empty((B, D), f)
    zf = np.empty((B,), f)
    lf = np.empty((B,), f)
    for c, r in enumerate(results):
        xf[c * PC : (c + 1) * PC] = r["xt_out"].T
        zl = r["zl_out"][0]
        zf[c * PC : (c + 1) * PC] = zl[:PC]
        lf[c * PC : (c + 1) * PC] = zl[PC:]
    return xf, zf, lf


def kernel(y0, ts, W1, b1, W2, b2, W3, b3):
    ts = np.asarray(ts, np.float32)
    key = ts.tobytes()
    if key not in _CACHE:
        _CACHE[key] = _build(ts)
    nc = _CACHE[key]
    in_maps = _prepare_inputs(y0, ts, W1, b1, W2, b2, W3, b3)
    res = _run(nc, in_maps)
    return _assemble(res.results)
